# revision 1
# baseline (speedup 1.0000x reference)
"""BiLSTM encoder kernel for Trainium2 (8 NeuronCores, SPMD).

Problem: B=32, T=512, I=H=512 bidirectional LSTM (PyTorch gate order).
  out  [B, T, 2H]  = concat(fwd hidden states, rev hidden states)
  h_n  [2, B, H], c_n [2, B, H] final states.

Design
------
One SPMD program runs on all 8 cores; per-core *data* differs.
Core 0 computes the forward direction, core 1 the reverse direction
(on host-reversed x).  Remaining cores run on duplicate data (ignored).

Per core:
  phase 1: gx[t] = W_ih @ x_t + (b_ih + b_hh) for all t as one big GEMM
           (lhsT = W_ih^T tiles stationary, x^T streamed), written to a
           DRAM scratch in [T, 128, (m,b)] layout.
  phase 2: 512 sequential steps.  Gates kept in "packed PSUM" layout:
           psum block [128 part, J*32] per gate-type block, partition =
           gate-row-within-chunk, free = (hid-chunk j, batch b).
           Gate blocks host-reordered to [g, i, f, o] so tanh(g) and
           sigmoid(i) are ready early and the c/h update overlaps the
           remaining matmuls.
           h is produced in [128 part(hid), (j,b)] layout which IS the
           transposed rhs the next step's matmul needs -> no transposes.

dtypes: W/x/h in bf16 (matmul inputs; FWL doubles weight-load rate),
        gx/psum/c/out in fp32.
"""

import os
import sys
import numpy as np

sys.path.insert(0, "/opt/trn_rl_repo")

import ml_dtypes  # noqa: E402

B = 32
T = 512
I_DIM = 512
H = 512
K4 = 4          # contraction chunks of 128 (both I and H are 512)
MS = 16         # gate-row chunks of 128 (4H = 2048)
J = 4           # hidden chunks of 128 (H = 512)
HW = J * 32     # free width of one gate block = 128

# PyTorch gate order in weights is [i, f, g, o]; we reorder rows to
# [g, i, f, o]: block 0 gets tanh, blocks 1..3 get sigmoid.
_PERM = np.concatenate([
    np.arange(1024, 1536),   # g
    np.arange(0, 512),       # i
    np.arange(512, 1024),    # f
    np.arange(1536, 2048),   # o
])

_BF16 = ml_dtypes.bfloat16

_PROGRAM_CACHE = {}


def _build_program(t_steps):
    import concourse.bass as bass  # noqa: F401
    import concourse.bacc as bacc
    import concourse.tile as tile
    import concourse.mybir as mybir
    from contextlib import ExitStack

    dt = mybir.dt
    AF = mybir.ActivationFunctionType

    nc = bacc.Bacc(
        "TRN2",
        target_bir_lowering=False,
        debug=False,
        enable_asserts=False,
        num_devices=8,
    )

    wih = nc.dram_tensor("wih", [128, K4 * MS * 128], dt.bfloat16, kind="ExternalInput")
    whh = nc.dram_tensor("whh", [128, K4 * MS * 128], dt.bfloat16, kind="ExternalInput")
    bias = nc.dram_tensor("bias", [128, MS], dt.float32, kind="ExternalInput")
    xT = nc.dram_tensor("xT", [K4, 128, t_steps * B], dt.bfloat16, kind="ExternalInput")
    h0 = nc.dram_tensor("h0", [128, J * 32], dt.bfloat16, kind="ExternalInput")
    c0 = nc.dram_tensor("c0", [128, J * 32], dt.float32, kind="ExternalInput")
    outs = nc.dram_tensor("outs", [t_steps, 128, J * 32], dt.float32, kind="ExternalOutput")
    clast = nc.dram_tensor("clast", [128, J * 32], dt.float32, kind="ExternalOutput")

    with tile.TileContext(nc) as tc, ExitStack() as ctx:
        const = ctx.enter_context(tc.tile_pool(name="const", bufs=1))
        dram = ctx.enter_context(tc.tile_pool(name="dram", bufs=1, space="DRAM"))

        w_ih_sb = const.tile([128, K4 * MS * 128], dt.bfloat16)
        w_hh_sb = const.tile([128, K4 * MS * 128], dt.bfloat16)
        bias_sb = const.tile([128, MS], dt.float32)
        h_bf = const.tile([128, J * 32], dt.bfloat16)
        c_t = const.tile([128, J * 32], dt.float32)

        nc.sync.dma_start(w_ih_sb[:], wih[:])
        nc.sync.dma_start(w_hh_sb[:], whh[:])
        nc.sync.dma_start(bias_sb[:], bias[:])
        nc.sync.dma_start(h_bf[:], h0[:])
        nc.sync.dma_start(c_t[:], c0[:])

        gx = dram.tile([t_steps, 128, MS * 32], dt.float32)

        # ---------------- phase 1: gx = W_ih @ x + bias ----------------
        n_chunks = (t_steps * B) // 512  # 512 cols = 16 t * 32 b
        with (
            tc.tile_pool(name="xp", bufs=3) as xpool,
            tc.tile_pool(name="ps1", bufs=4, space="PSUM") as ph1ps,
            tc.tile_pool(name="gxo", bufs=3) as gxout,
        ):
            for n in range(n_chunks):
                xtiles = []
                for k in range(K4):
                    xt_ = xpool.tile([128, 512], dt.bfloat16, tag=f"x{k}")
                    nc.sync.dma_start(xt_[:], xT[k, :, n * 512:(n + 1) * 512])
                    xtiles.append(xt_)
                for m in range(MS):
                    ps = ph1ps.tile([128, 512], dt.float32, tag="ps1")
                    for k in range(K4):
                        nc.tensor.matmul(
                            ps[:],
                            w_ih_sb[:, (k * MS + m) * 128:(k * MS + m + 1) * 128],
                            xtiles[k][:],
                            start=(k == 0),
                            stop=(k == K4 - 1),
                        )
                    gxt = gxout.tile([128, 512], dt.float32, tag="gxo")
                    nc.vector.tensor_scalar_add(gxt[:], ps[:], bias_sb[:, m:m + 1])
                    nc.sync.dma_start(
                        gx[n * 16:(n + 1) * 16, :, m * 32:(m + 1) * 32]
                        .rearrange("t p b -> p t b"),
                        gxt[:].rearrange("p (t b) -> p t b", t=16),
                    )

        # ---------------- phase 2: the recurrence ----------------
        with (
            tc.tile_pool(name="gxin", bufs=4) as gxin,
            tc.tile_pool(name="ps2", bufs=2, space="PSUM") as ps2,
            tc.tile_pool(name="work", bufs=2) as work,
        ):
            for t in range(t_steps):
                gxt = gxin.tile([128, MS * 32], dt.float32, tag="gxt")
                nc.sync.dma_start(gxt[:], gx[t])

                acts = []
                for blk in range(4):  # g, i, f, o
                    ps = ps2.tile([128, HW], dt.float32, tag=f"psb{blk}")
                    for mj in range(J):
                        m = blk * J + mj
                        for k in range(K4):
                            nc.tensor.matmul(
                                ps[:, mj * 32:(mj + 1) * 32],
                                w_hh_sb[:, (k * MS + m) * 128:(k * MS + m + 1) * 128],
                                h_bf[:, k * 32:(k + 1) * 32],
                                start=(k == 0),
                                stop=(k == K4 - 1),
                            )
                    nc.vector.tensor_add(ps[:], ps[:], gxt[:, blk * HW:(blk + 1) * HW])
                    a = work.tile([128, HW], dt.float32, tag=f"act{blk}")
                    nc.scalar.activation(a[:], ps[:], AF.Tanh if blk == 0 else AF.Sigmoid)
                    acts.append(a)

                tg, si, sf, so = acts
                t1 = work.tile([128, HW], dt.float32, tag="t1")
                nc.vector.tensor_mul(t1[:], si[:], tg[:])
                nc.vector.tensor_mul(c_t[:], c_t[:], sf[:])
                nc.vector.tensor_add(c_t[:], c_t[:], t1[:])
                tanc = work.tile([128, HW], dt.float32, tag="tanc")
                nc.scalar.activation(tanc[:], c_t[:], AF.Tanh)
                hf = work.tile([128, HW], dt.float32, tag="hf")
                nc.vector.tensor_mul(hf[:], so[:], tanc[:])
                nc.vector.tensor_copy(h_bf[:], hf[:])
                nc.sync.dma_start(outs[t], hf[:])

        nc.sync.dma_start(clast[:], c_t[:])

    nc.compile()
    return nc


def _get_program(t_steps):
    if t_steps not in _PROGRAM_CACHE:
        _PROGRAM_CACHE[t_steps] = _build_program(t_steps)
    return _PROGRAM_CACHE[t_steps]


def _prep_direction(x_d, h0_d, c0_d, w_ih, w_hh, b_ih, b_hh, t_steps):
    """Host-side packing for one direction. x_d is already time-ordered for
    the direction (reversed for the backward pass)."""
    wih_r = np.asarray(w_ih)[_PERM]           # [2048, 512]
    whh_r = np.asarray(w_hh)[_PERM]
    bias_r = (np.asarray(b_ih) + np.asarray(b_hh))[_PERM]

    def pack_w(w):
        # -> [128 part(p), (k, m, c)] with w_pack[p, (k*MS+m)*128+c] = w[m*128+c, k*128+p]
        a = w.reshape(MS, 128, K4, 128)        # [m, c, k, p]
        return np.ascontiguousarray(
            a.transpose(3, 2, 0, 1).reshape(128, K4 * MS * 128)
        ).astype(_BF16)

    bias_pack = np.ascontiguousarray(bias_r.reshape(MS, 128).T).astype(np.float32)

    xT_pack = np.ascontiguousarray(
        np.asarray(x_d).transpose(2, 1, 0).reshape(K4, 128, t_steps * B)
    ).astype(_BF16)

    def pack_state(s):
        # [B, H] -> [128, (j, b)]
        a = np.asarray(s).T.reshape(J, 128, 32)  # [j, p, b]
        return np.ascontiguousarray(a.transpose(1, 0, 2).reshape(128, J * 32))

    return {
        "wih": pack_w(wih_r),
        "whh": pack_w(whh_r),
        "bias": bias_pack,
        "xT": xT_pack,
        "h0": pack_state(h0_d).astype(_BF16),
        "c0": pack_state(c0_d).astype(np.float32),
    }


def _unpack_states(hs):
    # [128, (j, b)] -> [B, H]
    return np.ascontiguousarray(
        hs.reshape(128, J, 32).transpose(2, 1, 0).reshape(B, H)
    )


def _run(inputs, t_steps=T, trace=False):
    from concourse.bass_utils import run_bass_kernel_spmd

    x = np.asarray(inputs["x"], dtype=np.float32)[:, :t_steps]
    h0 = np.asarray(inputs["h0"], dtype=np.float32)
    c0 = np.asarray(inputs["c0"], dtype=np.float32)

    in_f = _prep_direction(
        x, h0[0], c0[0],
        inputs["w_ih_f"], inputs["w_hh_f"], inputs["b_ih_f"], inputs["b_hh_f"],
        t_steps,
    )
    in_r = _prep_direction(
        x[:, ::-1], h0[1], c0[1],
        inputs["w_ih_r"], inputs["w_hh_r"], inputs["b_ih_r"], inputs["b_hh_r"],
        t_steps,
    )

    nc = _get_program(t_steps)
    in_maps = [in_f, in_r] + [in_f] * 6
    res = run_bass_kernel_spmd(
        nc, in_maps, core_ids=list(range(8)), trace=trace,
    )
    outs_f = res.results[0]["outs"]  # [T, 128, J*32]
    outs_r = res.results[1]["outs"]
    c_f = res.results[0]["clast"]
    c_r = res.results[1]["clast"]

    # [t, p, j, b] -> [b, t, j*128+p]
    def to_bth(o):
        return np.ascontiguousarray(
            o.reshape(t_steps, 128, J, 32).transpose(3, 0, 2, 1).reshape(B, t_steps, H)
        )

    out = np.empty((B, t_steps, 2 * H), dtype=np.float32)
    out[:, :, :H] = to_bth(outs_f)
    out[:, :, H:] = to_bth(outs_r)[:, ::-1]

    h_n = np.stack([
        _unpack_states(outs_f[-1]),
        _unpack_states(outs_r[-1]),
    ]).astype(np.float32)
    c_n = np.stack([
        _unpack_states(c_f),
        _unpack_states(c_r),
    ]).astype(np.float32)

    return (out, h_n, c_n), res


def kernel(**inputs):
    result, _ = _run(inputs, t_steps=T, trace=False)
    return result


# revision 4
# speedup vs baseline: 52.0407x; 52.0407x over previous
"""BiLSTM encoder kernel for Trainium2 (8 NeuronCores, SPMD).

Problem: B=32, T=512, I=H=512 bidirectional LSTM (PyTorch gate order).
  out [B, T, 2H] = concat(fwd, rev hidden states); h_n, c_n [2, B, H].

Design
------
One SPMD program on all 8 cores; per-core data differs. Core (d*4 + q)
handles direction d (0=fwd, 1=rev on host-reversed x) for batch quarter
q (8 of 32 rows). The time recurrence is inherently sequential and its
per-step cost is weight-load-bound (full W_hh must stream into the PE
array every step), so batch sharding keeps per-step cost flat while
parallelizing the input projection 4x and shrinking I/O 4x.

Per core:
  phase 1: gx[t] = W_ih @ x_t + (b_ih+b_hh) for its batch quarter as one
           big GEMM (W_ih^T tiles stationary, x^T streamed), to DRAM
           scratch in [T, 128, (m-chunk, b)] layout.
  phase 2: 512 sequential steps, gates in packed-PSUM layout
           [128 gate-rows x (hid-chunk j, batch b)] per gate block; gate
           blocks host-reordered to [g, i, f, o] so tanh(g)/sigmoid(i)
           complete while later blocks' matmuls still run. h is produced
           in [128(hid), (j,b)] layout == the transposed rhs the next
           step needs: zero transposes anywhere.

dtypes: W/x/h bf16 (matmul inputs; FWL gives 2x weight-load), gx/psum/
        c/out fp32.
"""

import sys
import numpy as np

sys.path.insert(0, "/opt/trn_rl_repo")

import ml_dtypes  # noqa: E402

B = 32
T = 512
I_DIM = 512
H = 512
K4 = 4           # contraction chunks of 128 (both I and H are 512)
MS = 16          # gate-row chunks of 128 (4H = 2048)
J = 4            # hidden chunks of 128 (H = 512)
BQ = 4           # batch quarters (cores per direction)
BL = B // BQ     # local batch rows per core = 8
HW = J * BL      # free width of one gate block = 32

# PyTorch gate order [i, f, g, o] -> reordered [g, i, f, o]
_PERM = np.concatenate([
    np.arange(1024, 1536),   # g
    np.arange(0, 512),       # i
    np.arange(512, 1024),    # f
    np.arange(1536, 2048),   # o
])

_BF16 = ml_dtypes.bfloat16

_PROGRAM_CACHE = {}


def _build_program(t_steps):
    import concourse.bass as bass  # noqa: F401
    import concourse.bacc as bacc
    import concourse.tile as tile
    import concourse.mybir as mybir
    from contextlib import ExitStack

    dt = mybir.dt
    AF = mybir.ActivationFunctionType

    nc = bacc.Bacc(
        "TRN2",
        target_bir_lowering=False,
        debug=False,
        enable_asserts=False,
        num_devices=8,
    )

    wih = nc.dram_tensor("wih", [128, K4 * MS * 128], dt.bfloat16, kind="ExternalInput")
    whh = nc.dram_tensor("whh", [128, K4 * MS * 128], dt.bfloat16, kind="ExternalInput")
    bias = nc.dram_tensor("bias", [128, MS], dt.float32, kind="ExternalInput")
    xT = nc.dram_tensor("xT", [K4, 128, t_steps * BL], dt.bfloat16, kind="ExternalInput")
    h0 = nc.dram_tensor("h0", [128, J * BL], dt.bfloat16, kind="ExternalInput")
    c0 = nc.dram_tensor("c0", [128, J * BL], dt.float32, kind="ExternalInput")
    outs = nc.dram_tensor("outs", [t_steps, 128, J * BL], dt.float32, kind="ExternalOutput")
    clast = nc.dram_tensor("clast", [128, J * BL], dt.float32, kind="ExternalOutput")

    with tile.TileContext(nc) as tc, ExitStack() as ctx:
        const = ctx.enter_context(tc.tile_pool(name="const", bufs=1))
        dram = ctx.enter_context(tc.tile_pool(name="dram", bufs=1, space="DRAM"))

        w_ih_sb = const.tile([128, K4 * MS * 128], dt.bfloat16)
        w_hh_sb = const.tile([128, K4 * MS * 128], dt.bfloat16)
        bias_sb = const.tile([128, MS], dt.float32)
        h_bf = const.tile([128, J * BL], dt.bfloat16)
        c_t = const.tile([128, J * BL], dt.float32)

        nc.sync.dma_start(w_ih_sb[:], wih[:])
        nc.sync.dma_start(w_hh_sb[:], whh[:])
        nc.sync.dma_start(bias_sb[:], bias[:])
        nc.sync.dma_start(h_bf[:], h0[:])
        nc.sync.dma_start(c_t[:], c0[:])

        gx = dram.tile([t_steps, 128, MS * BL], dt.float32)

        # ---------------- phase 1: gx = W_ih @ x + bias ----------------
        ncols = min(512, t_steps * BL)       # columns per n-chunk
        tb = ncols // BL                     # timesteps per n-chunk
        n_chunks = (t_steps * BL) // ncols
        with (
            tc.tile_pool(name="xp", bufs=3) as xpool,
            tc.tile_pool(name="ps1", bufs=4, space="PSUM") as ph1ps,
            tc.tile_pool(name="gxo", bufs=3) as gxout,
        ):
            for n in range(n_chunks):
                xtiles = []
                for k in range(K4):
                    xt_ = xpool.tile([128, ncols], dt.bfloat16, tag=f"x{k}")
                    nc.sync.dma_start(xt_[:], xT[k, :, n * ncols:(n + 1) * ncols])
                    xtiles.append(xt_)
                for m in range(MS):
                    ps = ph1ps.tile([128, ncols], dt.float32, tag="ps1")
                    for k in range(K4):
                        nc.tensor.matmul(
                            ps[:],
                            w_ih_sb[:, (k * MS + m) * 128:(k * MS + m + 1) * 128],
                            xtiles[k][:],
                            start=(k == 0),
                            stop=(k == K4 - 1),
                        )
                    gxt = gxout.tile([128, ncols], dt.float32, tag="gxo")
                    nc.vector.tensor_scalar_add(gxt[:], ps[:], bias_sb[:, m:m + 1])
                    nc.sync.dma_start(
                        gx[n * tb:(n + 1) * tb, :, m * BL:(m + 1) * BL]
                        .rearrange("t p b -> p t b"),
                        gxt[:].rearrange("p (t b) -> p t b", t=tb),
                    )

        # ---------------- phase 2: the recurrence ----------------
        with (
            tc.tile_pool(name="gxin", bufs=4) as gxin,
            tc.tile_pool(name="ps2", bufs=2, space="PSUM") as ps2,
            tc.tile_pool(name="work", bufs=2) as work,
        ):
            for t in range(t_steps):
                gxt = gxin.tile([128, MS * BL], dt.float32, tag="gxt")
                nc.sync.dma_start(gxt[:], gx[t])

                acts = []
                for blk in range(4):  # g, i, f, o
                    ps = ps2.tile([128, HW], dt.float32, tag=f"psb{blk}")
                    for mj in range(J):
                        m = blk * J + mj
                        for k in range(K4):
                            nc.tensor.matmul(
                                ps[:, mj * BL:(mj + 1) * BL],
                                w_hh_sb[:, (k * MS + m) * 128:(k * MS + m + 1) * 128],
                                h_bf[:, k * BL:(k + 1) * BL],
                                start=(k == 0),
                                stop=(k == K4 - 1),
                            )
                    nc.vector.tensor_add(ps[:], ps[:], gxt[:, blk * HW:(blk + 1) * HW])
                    a = work.tile([128, HW], dt.float32, tag=f"act{blk}")
                    nc.scalar.activation(a[:], ps[:], AF.Tanh if blk == 0 else AF.Sigmoid)
                    acts.append(a)

                tg, si, sf, so = acts
                t1 = work.tile([128, HW], dt.float32, tag="t1")
                nc.vector.tensor_mul(t1[:], si[:], tg[:])
                nc.vector.tensor_mul(c_t[:], c_t[:], sf[:])
                nc.vector.tensor_add(c_t[:], c_t[:], t1[:])
                tanc = work.tile([128, HW], dt.float32, tag="tanc")
                nc.scalar.activation(tanc[:], c_t[:], AF.Tanh)
                hf = work.tile([128, HW], dt.float32, tag="hf")
                nc.vector.tensor_mul(hf[:], so[:], tanc[:])
                nc.vector.tensor_copy(h_bf[:], hf[:])
                nc.sync.dma_start(outs[t], hf[:])

        nc.sync.dma_start(clast[:], c_t[:])

    nc.compile()
    return nc


def _get_program(t_steps):
    if t_steps not in _PROGRAM_CACHE:
        _PROGRAM_CACHE[t_steps] = _build_program(t_steps)
    return _PROGRAM_CACHE[t_steps]


def _prep_weights(w_ih, w_hh, b_ih, b_hh):
    wih_r = np.asarray(w_ih)[_PERM]           # [2048, 512]
    whh_r = np.asarray(w_hh)[_PERM]
    bias_r = (np.asarray(b_ih) + np.asarray(b_hh))[_PERM]

    def pack_w(w):
        # [p, (k*MS+m)*128+c] = w[m*128+c, k*128+p]
        a = w.reshape(MS, 128, K4, 128)        # [m, c, k, p]
        return np.ascontiguousarray(
            a.transpose(3, 2, 0, 1).reshape(128, K4 * MS * 128)
        ).astype(_BF16)

    bias_pack = np.ascontiguousarray(bias_r.reshape(MS, 128).T).astype(np.float32)
    return pack_w(wih_r), pack_w(whh_r), bias_pack


def _prep_core(x_q, h0_q, c0_q, wpack, t_steps):
    """Pack per-core inputs: batch-quarter x [BL,T,I], states [BL,H]."""
    wih_p, whh_p, bias_p = wpack

    xT_pack = np.ascontiguousarray(
        np.asarray(x_q).transpose(2, 1, 0).reshape(K4, 128, t_steps * BL)
    ).astype(_BF16)

    def pack_state(s):
        a = np.asarray(s).T.reshape(J, 128, BL)  # [j, p, b]
        return np.ascontiguousarray(a.transpose(1, 0, 2).reshape(128, J * BL))

    return {
        "wih": wih_p,
        "whh": whh_p,
        "bias": bias_p,
        "xT": xT_pack,
        "h0": pack_state(h0_q).astype(_BF16),
        "c0": pack_state(c0_q).astype(np.float32),
    }


def _run(inputs, t_steps=T, trace=False):
    from concourse.bass_utils import run_bass_kernel_spmd

    x = np.asarray(inputs["x"], dtype=np.float32)[:, :t_steps]
    h0 = np.asarray(inputs["h0"], dtype=np.float32)
    c0 = np.asarray(inputs["c0"], dtype=np.float32)

    wf = _prep_weights(inputs["w_ih_f"], inputs["w_hh_f"],
                       inputs["b_ih_f"], inputs["b_hh_f"])
    wr = _prep_weights(inputs["w_ih_r"], inputs["w_hh_r"],
                       inputs["b_ih_r"], inputs["b_hh_r"])
    xr = x[:, ::-1]

    in_maps = []
    for q in range(BQ):
        sl = slice(q * BL, (q + 1) * BL)
        in_maps.append(_prep_core(x[sl], h0[0, sl], c0[0, sl], wf, t_steps))
    for q in range(BQ):
        sl = slice(q * BL, (q + 1) * BL)
        in_maps.append(_prep_core(xr[sl], h0[1, sl], c0[1, sl], wr, t_steps))

    nc = _get_program(t_steps)
    res = run_bass_kernel_spmd(nc, in_maps, core_ids=list(range(8)), trace=trace)

    # per-core outs [T, 128, (j, b)] -> [BL, T, H]
    def to_bth(o):
        return np.ascontiguousarray(
            o.reshape(t_steps, 128, J, BL).transpose(3, 0, 2, 1).reshape(BL, t_steps, H)
        )

    def state(o):
        return np.ascontiguousarray(
            o.reshape(128, J, BL).transpose(2, 1, 0).reshape(BL, H)
        )

    out = np.empty((B, t_steps, 2 * H), dtype=np.float32)
    h_n = np.empty((2, B, H), dtype=np.float32)
    c_n = np.empty((2, B, H), dtype=np.float32)
    for q in range(BQ):
        sl = slice(q * BL, (q + 1) * BL)
        of = res.results[q]
        orv = res.results[BQ + q]
        out[sl, :, :H] = to_bth(of["outs"])
        out[sl, :, H:] = to_bth(orv["outs"])[:, ::-1]
        h_n[0, sl] = state(of["outs"][-1])
        h_n[1, sl] = state(orv["outs"][-1])
        c_n[0, sl] = state(of["clast"])
        c_n[1, sl] = state(orv["clast"])

    return (out, h_n, c_n), res


def kernel(**inputs):
    result, _ = _run(inputs, t_steps=T, trace=False)
    return result


# revision 5
# speedup vs baseline: 55.5778x; 1.0680x over previous
"""BiLSTM encoder kernel for Trainium2 (8 NeuronCores, SPMD).

Problem: B=32, T=512, I=H=512 bidirectional LSTM (PyTorch gate order).
  out [B, T, 2H] = concat(fwd, rev hidden states); h_n, c_n [2, B, H].

Design
------
One SPMD program on all 8 cores; per-core data differs. Core (d*4 + q)
handles direction d (0=fwd, 1=rev on host-reversed x) for batch quarter
q (8 of 32 rows). The time recurrence is inherently sequential and its
per-step cost is weight-load-bound (full W_hh must stream into the PE
array every step), so batch sharding keeps per-step cost flat while
parallelizing the input projection 4x and shrinking I/O 4x.

Per core:
  phase 1: gx[t] = W_ih @ x_t + (b_ih+b_hh) for its batch quarter as one
           big GEMM (W_ih^T tiles stationary, x^T streamed), to DRAM
           scratch in [T, 128, (m-chunk, b)] layout.
  phase 2: 512 sequential steps, gates in packed-PSUM layout
           [128 gate-rows x (hid-chunk j, batch b)] per gate block; gate
           blocks host-reordered to [g, i, f, o] so tanh(g)/sigmoid(i)
           complete while later blocks' matmuls still run. h is produced
           in [128(hid), (j,b)] layout == the transposed rhs the next
           step needs: zero transposes anywhere.

dtypes: W/x/h bf16 (matmul inputs; FWL gives 2x weight-load), gx/psum/
        c/out fp32.
"""

import sys
import numpy as np

sys.path.insert(0, "/opt/trn_rl_repo")

import ml_dtypes  # noqa: E402

B = 32
T = 512
I_DIM = 512
H = 512
K4 = 4           # contraction chunks of 128 (both I and H are 512)
MS = 16          # gate-row chunks of 128 (4H = 2048)
J = 4            # hidden chunks of 128 (H = 512)
BQ = 4           # batch quarters (cores per direction)
BL = B // BQ     # local batch rows per core = 8
HW = J * BL      # free width of one gate block = 32

# PyTorch gate order [i, f, g, o] -> reordered [g, i, f, o]
_PERM = np.concatenate([
    np.arange(1024, 1536),   # g
    np.arange(0, 512),       # i
    np.arange(512, 1024),    # f
    np.arange(1536, 2048),   # o
])

_BF16 = ml_dtypes.bfloat16

_PROGRAM_CACHE = {}


def _build_program(t_steps):
    import concourse.bass as bass  # noqa: F401
    import concourse.bacc as bacc
    import concourse.tile as tile
    import concourse.mybir as mybir
    from contextlib import ExitStack

    dt = mybir.dt
    AF = mybir.ActivationFunctionType

    nc = bacc.Bacc(
        "TRN2",
        target_bir_lowering=False,
        debug=False,
        enable_asserts=False,
        num_devices=8,
    )

    wih = nc.dram_tensor("wih", [128, K4 * MS * 128], dt.bfloat16, kind="ExternalInput")
    whh = nc.dram_tensor("whh", [128, K4 * MS * 128], dt.bfloat16, kind="ExternalInput")
    bias = nc.dram_tensor("bias", [128, MS], dt.float32, kind="ExternalInput")
    xT = nc.dram_tensor("xT", [K4, 128, t_steps * BL], dt.bfloat16, kind="ExternalInput")
    h0 = nc.dram_tensor("h0", [128, J * BL], dt.bfloat16, kind="ExternalInput")
    c0 = nc.dram_tensor("c0", [128, J * BL], dt.float32, kind="ExternalInput")
    ident = nc.dram_tensor("ident", [128, 128], dt.float32, kind="ExternalInput")
    outs = nc.dram_tensor("outs", [t_steps, 128, J * BL], dt.float32, kind="ExternalOutput")
    clast = nc.dram_tensor("clast", [128, J * BL], dt.float32, kind="ExternalOutput")

    with tile.TileContext(nc) as tc, ExitStack() as ctx:
        const = ctx.enter_context(tc.tile_pool(name="const", bufs=1))
        dram = ctx.enter_context(tc.tile_pool(name="dram", bufs=1, space="DRAM"))

        w_ih_sb = const.tile([128, K4 * MS * 128], dt.bfloat16)
        w_hh_sb = const.tile([128, K4 * MS * 128], dt.bfloat16)
        bias_sb = const.tile([128, MS], dt.float32)
        ident_sb = const.tile([128, 128], dt.float32)
        h_bf = const.tile([128, J * BL], dt.bfloat16)
        c_t = const.tile([128, J * BL], dt.float32)

        nc.sync.dma_start(w_ih_sb[:], wih[:])
        nc.sync.dma_start(w_hh_sb[:], whh[:])
        nc.sync.dma_start(bias_sb[:], bias[:])
        nc.sync.dma_start(ident_sb[:], ident[:])
        nc.sync.dma_start(h_bf[:], h0[:])
        nc.sync.dma_start(c_t[:], c0[:])

        gx = dram.tile([t_steps, 128, MS * BL], dt.float32)

        # ---------------- phase 1: gx = W_ih @ x + bias ----------------
        ncols = min(512, t_steps * BL)       # columns per n-chunk
        tb = ncols // BL                     # timesteps per n-chunk
        n_chunks = (t_steps * BL) // ncols
        with (
            tc.tile_pool(name="xp", bufs=3) as xpool,
            tc.tile_pool(name="ps1", bufs=4, space="PSUM") as ph1ps,
            tc.tile_pool(name="gxo", bufs=3) as gxout,
        ):
            for n in range(n_chunks):
                xtiles = []
                for k in range(K4):
                    xt_ = xpool.tile([128, ncols], dt.bfloat16, tag=f"x{k}")
                    nc.sync.dma_start(xt_[:], xT[k, :, n * ncols:(n + 1) * ncols])
                    xtiles.append(xt_)
                for m in range(MS):
                    ps = ph1ps.tile([128, ncols], dt.float32, tag="ps1")
                    for k in range(K4):
                        nc.tensor.matmul(
                            ps[:],
                            w_ih_sb[:, (k * MS + m) * 128:(k * MS + m + 1) * 128],
                            xtiles[k][:],
                            start=(k == 0),
                            stop=(k == K4 - 1),
                        )
                    gxt = gxout.tile([128, ncols], dt.float32, tag="gxo")
                    nc.vector.tensor_scalar_add(gxt[:], ps[:], bias_sb[:, m:m + 1])
                    nc.sync.dma_start(
                        gx[n * tb:(n + 1) * tb, :, m * BL:(m + 1) * BL]
                        .rearrange("t p b -> p t b"),
                        gxt[:].rearrange("p (t b) -> p t b", t=tb),
                    )

        # ---------------- phase 2: the recurrence ----------------
        with (
            tc.tile_pool(name="gxin", bufs=4) as gxin,
            tc.tile_pool(name="ps2", bufs=2, space="PSUM") as ps2,
            tc.tile_pool(name="work", bufs=2) as work,
        ):
            for t in range(t_steps):
                gxt = gxin.tile([128, MS * BL], dt.float32, tag="gxt")
                nc.sync.dma_start(gxt[:], gx[t])

                acts = []
                for blk in range(4):  # g, i, f, o
                    ps = ps2.tile([128, HW], dt.float32, tag=f"psb{blk}")
                    # seed the accumulation with gx via identity matmul
                    nc.tensor.matmul(
                        ps[:],
                        ident_sb[:],
                        gxt[:, blk * HW:(blk + 1) * HW],
                        start=True,
                        stop=False,
                        skip_group_check=True,
                    )
                    for mj in range(J):
                        m = blk * J + mj
                        for k in range(K4):
                            nc.tensor.matmul(
                                ps[:, mj * BL:(mj + 1) * BL],
                                w_hh_sb[:, (k * MS + m) * 128:(k * MS + m + 1) * 128],
                                h_bf[:, k * BL:(k + 1) * BL],
                                start=False,
                                stop=(mj == J - 1 and k == K4 - 1),
                                skip_group_check=True,
                            )
                    a = work.tile([128, HW], dt.float32, tag=f"act{blk}")
                    nc.scalar.activation(a[:], ps[:], AF.Tanh if blk == 0 else AF.Sigmoid)
                    acts.append(a)

                tg, si, sf, so = acts
                t1 = work.tile([128, HW], dt.float32, tag="t1")
                nc.vector.tensor_mul(t1[:], si[:], tg[:])
                nc.vector.tensor_mul(c_t[:], c_t[:], sf[:])
                nc.vector.tensor_add(c_t[:], c_t[:], t1[:])
                tanc = work.tile([128, HW], dt.float32, tag="tanc")
                nc.scalar.activation(tanc[:], c_t[:], AF.Tanh)
                hf = work.tile([128, HW], dt.float32, tag="hf")
                nc.vector.tensor_mul(hf[:], so[:], tanc[:])
                nc.vector.tensor_copy(h_bf[:], hf[:])
                nc.sync.dma_start(outs[t], hf[:])

        nc.sync.dma_start(clast[:], c_t[:])

    nc.compile()
    return nc


def _get_program(t_steps):
    if t_steps not in _PROGRAM_CACHE:
        _PROGRAM_CACHE[t_steps] = _build_program(t_steps)
    return _PROGRAM_CACHE[t_steps]


def _prep_weights(w_ih, w_hh, b_ih, b_hh):
    wih_r = np.asarray(w_ih)[_PERM]           # [2048, 512]
    whh_r = np.asarray(w_hh)[_PERM]
    bias_r = (np.asarray(b_ih) + np.asarray(b_hh))[_PERM]

    def pack_w(w):
        # [p, (k*MS+m)*128+c] = w[m*128+c, k*128+p]
        a = w.reshape(MS, 128, K4, 128)        # [m, c, k, p]
        return np.ascontiguousarray(
            a.transpose(3, 2, 0, 1).reshape(128, K4 * MS * 128)
        ).astype(_BF16)

    bias_pack = np.ascontiguousarray(bias_r.reshape(MS, 128).T).astype(np.float32)
    return pack_w(wih_r), pack_w(whh_r), bias_pack


def _prep_core(x_q, h0_q, c0_q, wpack, t_steps):
    """Pack per-core inputs: batch-quarter x [BL,T,I], states [BL,H]."""
    wih_p, whh_p, bias_p = wpack

    xT_pack = np.ascontiguousarray(
        np.asarray(x_q).transpose(2, 1, 0).reshape(K4, 128, t_steps * BL)
    ).astype(_BF16)

    def pack_state(s):
        a = np.asarray(s).T.reshape(J, 128, BL)  # [j, p, b]
        return np.ascontiguousarray(a.transpose(1, 0, 2).reshape(128, J * BL))

    return {
        "ident": np.eye(128, dtype=np.float32),
        "wih": wih_p,
        "whh": whh_p,
        "bias": bias_p,
        "xT": xT_pack,
        "h0": pack_state(h0_q).astype(_BF16),
        "c0": pack_state(c0_q).astype(np.float32),
    }


def _run(inputs, t_steps=T, trace=False):
    from concourse.bass_utils import run_bass_kernel_spmd

    x = np.asarray(inputs["x"], dtype=np.float32)[:, :t_steps]
    h0 = np.asarray(inputs["h0"], dtype=np.float32)
    c0 = np.asarray(inputs["c0"], dtype=np.float32)

    wf = _prep_weights(inputs["w_ih_f"], inputs["w_hh_f"],
                       inputs["b_ih_f"], inputs["b_hh_f"])
    wr = _prep_weights(inputs["w_ih_r"], inputs["w_hh_r"],
                       inputs["b_ih_r"], inputs["b_hh_r"])
    xr = x[:, ::-1]

    in_maps = []
    for q in range(BQ):
        sl = slice(q * BL, (q + 1) * BL)
        in_maps.append(_prep_core(x[sl], h0[0, sl], c0[0, sl], wf, t_steps))
    for q in range(BQ):
        sl = slice(q * BL, (q + 1) * BL)
        in_maps.append(_prep_core(xr[sl], h0[1, sl], c0[1, sl], wr, t_steps))

    nc = _get_program(t_steps)
    res = run_bass_kernel_spmd(nc, in_maps, core_ids=list(range(8)), trace=trace)

    # per-core outs [T, 128, (j, b)] -> [BL, T, H]
    def to_bth(o):
        return np.ascontiguousarray(
            o.reshape(t_steps, 128, J, BL).transpose(3, 0, 2, 1).reshape(BL, t_steps, H)
        )

    def state(o):
        return np.ascontiguousarray(
            o.reshape(128, J, BL).transpose(2, 1, 0).reshape(BL, H)
        )

    out = np.empty((B, t_steps, 2 * H), dtype=np.float32)
    h_n = np.empty((2, B, H), dtype=np.float32)
    c_n = np.empty((2, B, H), dtype=np.float32)
    for q in range(BQ):
        sl = slice(q * BL, (q + 1) * BL)
        of = res.results[q]
        orv = res.results[BQ + q]
        out[sl, :, :H] = to_bth(of["outs"])
        out[sl, :, H:] = to_bth(orv["outs"])[:, ::-1]
        h_n[0, sl] = state(of["outs"][-1])
        h_n[1, sl] = state(orv["outs"][-1])
        c_n[0, sl] = state(of["clast"])
        c_n[1, sl] = state(orv["clast"])

    return (out, h_n, c_n), res


def kernel(**inputs):
    result, _ = _run(inputs, t_steps=T, trace=False)
    return result
